# revision 1
# baseline (speedup 1.0000x reference)
"""Trainium2 Bass kernel for fused MHA block (nn_MultiHeadAttention_7636451852747).

Reference math (B=2, S=4096, D=512, H=8, hd=64):
    q = (x @ Wq + bq).reshape(B, H, S, hd)   # torch-style .view, no transpose!
    ... scores = q @ k^T / 8; attn = softmax(scores) @ v -> reshape(B,S,D)
    y = LayerNorm(x + attn) * gamma + beta

Key structural insight: the .view(B,H,S,hd) reshape (without transpose) means
head h of batch b only reads rows [h*512, (h+1)*512) of x[b].  The problem
therefore decomposes into B*H = 16 fully independent [512,512] chunks; each of
the 8 cores processes 2 chunks end-to-end with zero inter-core communication.

Within a chunk (x_c = x[b, h*512:(h+1)*512, :], shape [512, 512]):
    q = x_c Wq + bq viewed as Q[4096, 64] with Q[8s+j, d] = q[s, 64j+d]
    scores^T tiles: S_T[(jk,r)][p, s_q] for nk = 8*(128r+p)+jk, nq = 8*s_q+jq
      = matmul(lhsT=k^T[64jk:+64, 128r:+128], rhs=q^T[64jq:+64, :])
    E = exp(S_T/8) in bf16; attn^T strip = sum over (jk,r) of
      matmul(lhsT=[V_tile | ones], rhs=E) -> [65, 512] psum; row 64 = softmax
      denominator (ones-column trick).  PE-transpose [65,128] blocks back to
      natural layout, divide by denominator, add residual, LayerNorm on DVE
      (Newton rsqrt to keep ACT free for exp, which is the bottleneck engine).
All matmuls are bf16 with fp32 PSUM accumulation.
"""
import os
import numpy as np
import ml_dtypes
from contextlib import ExitStack

BF16 = None  # set in _imports
_STATE = {}


def _imports():
    global bass, bacc, tile, mybir, bass_utils, F32, BF16, I32, ALU, ACTF
    import concourse.bass as bass
    import concourse.bacc as bacc
    import concourse.tile as tile
    from concourse import mybir
    from concourse import bass_utils
    F32 = mybir.dt.float32
    BF16 = mybir.dt.bfloat16
    I32 = mybir.dt.int32
    ALU = mybir.AluOpType
    ACTF = mybir.ActivationFunctionType


N_CORES = 8
CHUNKS_PER_CORE = 2
S = 512          # rows per chunk
D = 512          # model dim
HD = 64          # head dim of the viewed [4096, 64] matrices
NQ = 4096        # sub-rows per chunk (S*D/HD)
EPS = 1e-5


def _emit(nc, tc, ctx):
    F32l, BF16l, I32l = F32, BF16, I32
    x_d = nc.dram_tensor("xc", [CHUNKS_PER_CORE, S, D], F32l, kind="ExternalInput").ap()
    xb_d = nc.dram_tensor("xcb", [CHUNKS_PER_CORE, S, D], BF16l, kind="ExternalInput").ap()
    w_d = {n: nc.dram_tensor(n, [D, D], BF16l, kind="ExternalInput").ap()
           for n in ("wq", "wk", "wv")}
    b_d = {n: nc.dram_tensor(n, [1, D], BF16l, kind="ExternalInput").ap()
           for n in ("bq", "bk", "bv")}
    ones_d = nc.dram_tensor("ones", [1, D], BF16l, kind="ExternalInput").ap()
    idf_d = nc.dram_tensor("idf", [128, 128], F32l, kind="ExternalInput").ap()
    gb_d = nc.dram_tensor("gb", [128, D], F32l, kind="ExternalInput").ap()
    bb_d = nc.dram_tensor("bb", [128, D], F32l, kind="ExternalInput").ap()
    y_d = nc.dram_tensor("y", [CHUNKS_PER_CORE, S, D], F32l, kind="ExternalOutput").ap()

    # pools
    consts = ctx.enter_context(tc.tile_pool(name="consts", bufs=1))
    chunkp = ctx.enter_context(tc.tile_pool(name="chunk", bufs=2))
    epool = ctx.enter_context(tc.tile_pool(name="epool", bufs=8))
    attp = ctx.enter_context(tc.tile_pool(name="attp", bufs=2))
    ypool = ctx.enter_context(tc.tile_pool(name="ypool", bufs=3))
    small = ctx.enter_context(tc.tile_pool(name="small", bufs=4))
    # PSUM budget (8 banks): score 2x[128,1024]=4, attn 2, proj 2 (shared
    # with the finalize transposes via the same tag)
    ps_proj = ctx.enter_context(tc.tile_pool(name="ps_proj", bufs=2, space="PSUM"))
    ps_score = ctx.enter_context(tc.tile_pool(name="ps_score", bufs=2, space="PSUM"))
    ps_attn = ctx.enter_context(tc.tile_pool(name="ps_attn", bufs=2, space="PSUM"))

    # ---- constant tiles (DMAs emitted by _consts_early/_late below so the
    # x-transpose DMAs can go FIRST in the single HWDGE queue: the first
    # projection matmul is gated on x^T, not on the weights)
    w_sb = {n: consts.tile([128, 4 * D], BF16l, tag=n, name=f"w_{n}")
            for n in ("wq", "wk", "wv")}
    b_sb = {n: consts.tile([1, D], BF16l, tag=n, name=f"b_{n}")
            for n in ("bq", "bk", "bv")}
    ones = consts.tile([1, D], BF16l, tag="ones")
    idf = consts.tile([128, 128], F32l, tag="idf")
    gb = consts.tile([128, D], F32l, tag="gb")
    bb = consts.tile([128, D], F32l, tag="bb")

    def consts_early():
        for n in ("wq", "wk"):
            for mt in range(4):
                nc.sync.dma_start(w_sb[n][:, 512 * mt:512 * (mt + 1)],
                                  w_d[n][128 * mt:128 * (mt + 1), :])
        for n in ("bq", "bk"):
            nc.sync.dma_start(b_sb[n][:], b_d[n][:])
        nc.sync.dma_start(ones[:], ones_d[:])

    def consts_late():
        for mt in range(4):
            nc.sync.dma_start(w_sb["wv"][:, 512 * mt:512 * (mt + 1)],
                              w_d["wv"][128 * mt:128 * (mt + 1), :])
        nc.sync.dma_start(b_sb["bv"][:], b_d["bv"][:])
        nc.sync.dma_start(idf[:], idf_d[:])
        nc.sync.dma_start(gb[:], gb_d[:])
        nc.sync.dma_start(bb[:], bb_d[:])

    st = [{} for _ in range(CHUNKS_PER_CORE)]  # per-chunk tile state

    def prep_load(c):
        """DMA x; x^T in one hardware DMA transpose (XBAR, bf16).
        dma_start_transpose into a [p, mt, s] view lands source row m at
        partition m%128 of slab m//128 -- exactly the m-tile-major layout."""
        s = st[c]
        s["xT"] = xT = chunkp.tile([128, 4 * D], BF16l, tag="xT", name=f"xT{c}")
        for mt in range(4):
            nc.sync.dma_start_transpose(
                xT[:, 512 * mt:512 * (mt + 1)], xb_d[c][:, 128 * mt:128 * (mt + 1)])
        s["xf"] = xf = chunkp.tile([128, 4 * D], F32l, tag="xf", name=f"xf{c}")
        for t in range(4):
            nc.sync.dma_start(xf[:, 512 * t:512 * (t + 1)], x_d[c, 128 * t:128 * (t + 1), :])
        s["qT"] = chunkp.tile([128, 4 * D], BF16l, tag="qT", name=f"qT{c}")
        s["qTs"] = chunkp.tile([128, 4 * D], BF16l, tag="qTs", name=f"qTs{c}")
        s["kT"] = chunkp.tile([128, 4 * D], BF16l, tag="kT", name=f"kT{c}")
        s["vp"] = chunkp.tile([128, 4 * 520], BF16l, tag="vp", name=f"vp{c}")
        s["h"] = chunkp.tile([128, 4 * D], F32l, tag="h", name=f"h{c}")

    def prep_qk(c, t, which):
        """One q^T or k^T projection column tile (plus qTs swap for q)."""
        s = st[c]
        xT, qT, qTs, kT = s["xT"], s["qT"], s["qTs"], s["kT"]
        wname, bname, dst = (("wq", "bq", qT) if which == "q" else ("wk", "bk", kT))
        pp = ps_proj.tile([128, D], F32l, tag="proj", name=f"pp{c}_{wname}{t}")
        for mt in range(4):
            nc.tensor.matmul(
                pp[:],
                w_sb[wname][:, 512 * mt + 128 * t:512 * mt + 128 * t + 128],
                xT[:, 512 * mt:512 * (mt + 1)],
                start=(mt == 0), stop=False)
        nc.tensor.matmul(pp[:], b_sb[bname][0:1, 128 * t:128 * (t + 1)],
                         ones[0:1, :], start=False, stop=True)
        nc.vector.tensor_copy(dst[0:64, 512 * t:512 * (t + 1)], pp[0:64, :])
        nc.vector.tensor_copy(dst[64:128, 512 * t:512 * (t + 1)], pp[64:128, :])
        if which == "q":
            nc.sync.dma_start(qTs[64:128, 512 * t:512 * (t + 1)], qT[0:64, 512 * t:512 * (t + 1)])
            nc.sync.dma_start(qTs[0:64, 512 * t:512 * (t + 1)], qT[64:128, 512 * t:512 * (t + 1)])

    def prep_v(c, t):
        s = st[c]
        xT, vp = s["xT"], s["vp"]
        pp = ps_proj.tile([128, D], F32l, tag="proj", name=f"pp{c}_v{t}")
        for mt in range(4):
            nc.tensor.matmul(pp[:], xT[:, 512 * mt + 128 * t:512 * mt + 128 * t + 128],
                             w_sb["wv"][:, 512 * mt:512 * (mt + 1)],
                             start=(mt == 0), stop=False)
        nc.tensor.matmul(pp[:], ones[0:1, 0:128], b_sb["bv"][0:1, :],
                         start=False, stop=True)
        blk = vp[:, 520 * t:520 * (t + 1)].rearrange("p (j c) -> p j c", c=65)
        nc.vector.tensor_copy(blk[:, :, 0:64], pp[:].rearrange("p (j c) -> p j c", c=64))
        nc.vector.memset(blk[:, :, 64], 1.0)

    def prep_qkv(c, t):
        prep_qk(c, t, "q")
        prep_qk(c, t, "k")
        prep_v(c, t)

    def strips(c, jp):
        """One jq-pair: scores (row-packed), 1024-wide exp, attn accumulate,
        transpose back + residual."""
        s = st[c]
        qT, qTs, kT, vp, xf, h = s["qT"], s["qTs"], s["kT"], s["vp"], s["xf"], s["h"]

        def qrhs(jq, par):
            src = qT if (jq % 2) == par else qTs
            return src[64 * par:64 * par + 64, 512 * (jq // 2):512 * (jq // 2) + 512]

        jq0, jq1 = 2 * jp, 2 * jp + 1
        pa = [ps_attn.tile([65, D], F32l, tag="attn", name=f"pa{c}_{jp}_{i}")
              for i in range(2)]
        for r in range(4):
            for jku in range(4):
                jk0, jk1 = 2 * jku, 2 * jku + 1
                koff = 512 * jku + 128 * r
                ps0 = ps_score.tile([128, 2 * D], F32l, tag="sps", name=f"s0_{c}_{jp}_{r}_{jku}")
                ps1 = ps_score.tile([128, 2 * D], F32l, tag="sps", name=f"s1_{c}_{jp}_{r}_{jku}")
                nc.tensor.matmul(ps0[:, 0:512], kT[0:64, koff:koff + 128],
                                 qrhs(jq0, 0), start=True, stop=True,
                                 tile_position=(0, 0))
                nc.tensor.matmul(ps1[:, 0:512], kT[64:128, koff:koff + 128],
                                 qrhs(jq0, 1), start=True, stop=True,
                                 tile_position=(64, 0))
                nc.tensor.matmul(ps0[:, 512:1024], kT[0:64, koff:koff + 128],
                                 qrhs(jq1, 0), start=True, stop=True,
                                 tile_position=(0, 0))
                nc.tensor.matmul(ps1[:, 512:1024], kT[64:128, koff:koff + 128],
                                 qrhs(jq1, 1), start=True, stop=True,
                                 tile_position=(64, 0))
                et0 = epool.tile([128, 2 * D], BF16l, tag="e", name=f"e0_{c}_{jp}_{r}_{jku}")
                et1 = epool.tile([128, 2 * D], BF16l, tag="e", name=f"e1_{c}_{jp}_{r}_{jku}")
                nc.scalar.activation(et0[:], ps0[:], ACTF.Exp, scale=0.125)
                nc.scalar.activation(et1[:], ps1[:], ACTF.Exp, scale=0.125)
                first = (r == 0 and jku == 0)
                last = (r == 3 and jku == 3)
                v0 = vp[:, 520 * r + 65 * jk0:520 * r + 65 * jk0 + 65]
                v1 = vp[:, 520 * r + 65 * jk1:520 * r + 65 * jk1 + 65]
                nc.tensor.matmul(pa[0][:], v0, et0[:, 0:512], start=first,
                                 stop=False, skip_group_check=True)
                nc.tensor.matmul(pa[0][:], v1, et1[:, 0:512], start=False,
                                 stop=last, skip_group_check=True)
                nc.tensor.matmul(pa[1][:], v0, et0[:, 512:1024], start=first,
                                 stop=False, skip_group_check=True)
                nc.tensor.matmul(pa[1][:], v1, et1[:, 512:1024], start=False,
                                 stop=last, skip_group_check=True)
        for half, jq in ((0, jq0), (1, jq1)):
            asb = attp.tile([65, D], F32l, tag="asb", name=f"asb{c}_{jp}_{half}")
            nc.vector.tensor_copy(asb[:], pa[half][:])
            for b in range(4):
                tps = ps_proj.tile([128, 65], F32l, tag="proj", name=f"atr{c}_{jp}_{half}_{b}")
                nc.tensor.transpose(tps[:], asb[0:65, 128 * b:128 * (b + 1)],
                                    idf[0:65, 0:65])
                rcp = small.tile([128, 1], F32l, tag="rcp", name=f"rcp{c}_{jp}_{half}_{b}")
                nc.vector.reciprocal(rcp[:], tps[:, 64:65])
                nc.vector.scalar_tensor_tensor(
                    h[:, 512 * b + 64 * jq:512 * b + 64 * jq + 64],
                    tps[:, 0:64], rcp[:],
                    xf[:, 512 * b + 64 * jq:512 * b + 64 * jq + 64],
                    op0=ALU.mult, op1=ALU.add)

    def layer_norm(c):
        """LayerNorm on DVE only; Newton rsqrt batched across the 4 s-tiles."""
        s = st[c]
        h = s["h"]
        mvall = small.tile([128, 8], F32l, tag="mvall", name=f"mv{c}")
        for b in range(4):
            st6 = small.tile([128, 6], F32l, tag="st6", name=f"st6_{c}_{b}")
            nc.vector.bn_stats(st6[:], h[:, 512 * b:512 * (b + 1)])
            nc.vector.bn_aggr(mvall[:, 2 * b:2 * b + 2], st6[:])
        mean4 = mvall[:].rearrange("p (b two) -> p b two", two=2)[:, :, 0]
        var4 = mvall[:].rearrange("p (b two) -> p b two", two=2)[:, :, 1]
        t4 = small.tile([128, 4], F32l, tag="t4", name=f"t4_{c}")
        nc.vector.tensor_scalar_add(t4[:], var4, EPS)
        yi = small.tile([128, 4], I32l, tag="yi", name=f"yi{c}")
        nc.vector.tensor_scalar(yi[:], t4[:].bitcast(I32l), 1, None,
                                op0=ALU.arith_shift_right)
        nc.vector.tensor_scalar(yi[:], yi[:], 0x5F3759DF, -1,
                                op0=ALU.subtract, op1=ALU.mult)
        rstd = small.tile([128, 4], F32l, tag="rstd", name=f"rstd{c}")
        nc.vector.tensor_copy(rstd[:], yi[:].bitcast(F32l))
        y2 = small.tile([128, 4], F32l, tag="y2", name=f"y2_{c}")
        dd = small.tile([128, 4], F32l, tag="dd", name=f"dd{c}")
        for _ in range(3):
            nc.vector.tensor_tensor(y2[:], rstd[:], rstd[:], op=ALU.mult)
            nc.vector.tensor_tensor(y2[:], y2[:], t4[:], op=ALU.mult)
            nc.vector.tensor_scalar(dd[:], y2[:], -0.5, 1.5,
                                    op0=ALU.mult, op1=ALU.add)
            nc.vector.tensor_tensor(rstd[:], rstd[:], dd[:], op=ALU.mult)
        bco = small.tile([128, 4], F32l, tag="bco", name=f"bco{c}")
        nc.vector.tensor_tensor(bco[:], mean4, rstd[:], op=ALU.mult)
        nc.vector.tensor_scalar_mul(bco[:], bco[:], -1.0)
        for b in range(4):
            yt = ypool.tile([128, D], F32l, tag="yt", name=f"yt{c}_{b}")
            nc.vector.tensor_scalar(yt[:], h[:, 512 * b:512 * (b + 1)],
                                    rstd[:, b:b + 1], bco[:, b:b + 1],
                                    op0=ALU.mult, op1=ALU.add)
            nc.vector.tensor_tensor(yt[:], yt[:], gb[:], op=ALU.mult)
            nc.vector.tensor_tensor(yt[:], yt[:], bb[:], op=ALU.add)
            nc.sync.dma_start(y_d[c, 128 * b:128 * (b + 1), :], yt[:])

    # ---- emission schedule: stagger chunk-1 prep into chunk-0's strips so
    # the PE fills ACT-idle gaps with the next chunk's projections.
    prep_load(0)
    consts_early()
    consts_late()
    for t in range(4):
        prep_qkv(0, t)
    strips(0, 0)
    prep_load(1)
    strips(0, 1)
    prep_qk(1, 0, "q")
    prep_qk(1, 0, "k")
    prep_v(1, 0)
    prep_qk(1, 1, "q")
    strips(0, 2)
    prep_qk(1, 1, "k")
    prep_v(1, 1)
    prep_qk(1, 2, "q")
    prep_qk(1, 2, "k")
    strips(0, 3)
    prep_v(1, 2)
    prep_qk(1, 3, "q")
    prep_qk(1, 3, "k")
    prep_v(1, 3)
    layer_norm(0)
    for jp in range(4):
        strips(1, jp)
    layer_norm(1)


def build():
    """Build + compile the Bass module (cached)."""
    if "nc" in _STATE:
        return _STATE["nc"]
    _imports()
    nc = bacc.Bacc("TRN2", target_bir_lowering=False, debug=False,
                   num_devices=N_CORES)
    with tile.TileContext(nc) as tc:
        with ExitStack() as ctx:
            _emit(nc, tc, ctx)
    nc.compile()
    _STATE["nc"] = nc
    return nc


def host_inputs(Wq, bq, Wk, bk, Wv, bv, gamma, beta):
    """Shared per-core constant inputs (everything except x chunks)."""
    bf = ml_dtypes.bfloat16
    base = {
        "wq": np.asarray(Wq, np.float32).astype(bf),
        "wk": np.asarray(Wk, np.float32).astype(bf),
        "wv": np.asarray(Wv, np.float32).astype(bf),
        "bq": np.asarray(bq, np.float32).reshape(1, D).astype(bf),
        "bk": np.asarray(bk, np.float32).reshape(1, D).astype(bf),
        "bv": np.asarray(bv, np.float32).reshape(1, D).astype(bf),
        "ones": np.ones((1, D), bf),
        "idf": np.eye(128, dtype=np.float32),
        "gb": np.broadcast_to(np.asarray(gamma, np.float32), (128, D)).copy(),
        "bb": np.broadcast_to(np.asarray(beta, np.float32), (128, D)).copy(),
    }
    return base


def kernel(x, Wq, bq, Wk, bk, Wv, bv, gamma, beta):
    _imports()
    nc = build()
    x = np.asarray(x, np.float32)
    B, Sfull, Dm = x.shape
    chunks = x.reshape(B * 8, S, D)  # chunk c = (b = c//8, head = c%8)
    bf = ml_dtypes.bfloat16
    base = host_inputs(Wq=Wq, bq=bq, Wk=Wk, bk=bk, Wv=Wv, bv=bv,
                       gamma=gamma, beta=beta)
    in_maps = []
    for i in range(N_CORES):
        xc = np.ascontiguousarray(chunks[2 * i:2 * i + 2])
        m = dict(base)
        m["xc"] = xc
        m["xcb"] = xc.astype(bf)
        in_maps.append(m)
    res = bass_utils.run_bass_kernel_spmd(nc, in_maps, core_ids=list(range(N_CORES)))
    out_chunks = np.empty((B * 8, S, D), np.float32)
    for i in range(N_CORES):
        out_chunks[2 * i:2 * i + 2] = res.results[i]["y"]
    return out_chunks.reshape(B, Sfull, Dm)



# revision 37
# speedup vs baseline: 1.7336x; 1.7336x over previous
"""Trainium2 Bass kernel for fused MHA block (nn_MultiHeadAttention_7636451852747).

Reference math (B=2, S=4096, D=512, H=8, hd=64):
    q = (x @ Wq).reshape(B, H, S, hd)   # torch-style .view, no transpose!
    scores = q @ k^T / 8; attn = softmax(scores) @ v -> reshape(B,S,D)
    y = LayerNorm(x + attn)

The .view reshape means head h of batch b only reads rows [512h, 512h+512) of
x[b]: the problem splits into B*H = 16 independent [512,512] chunks; each of 8
cores handles 2 chunks, no inter-core communication.

Per chunk (all on-device; host only reformats/casts inputs):
  * q^T,k^T projections in bf16 (PE, stationary W slabs, moving x^T),
    v projection in fp8e4 DoubleRow (half-cost matmuls).
  * scores^T tiles [128 keys, 2 i-blocks x 512 q-cols] per (jq, r, jku):
    2 bf16 matmuls packing both PE quadrants (kT halves + qT/qTs swap).
  * exp((s - 33)/8) into fp8e4, split across two engines: ACT native Exp
    (bias=-4.125, scale=0.125) and DVE one-op Schraudolph
    (uint8 = round(log2e*s + C), bitcast to e4m3; the -33 shift keeps the
    bit pattern inside the e4m3 NaN-free range; shift cancels in softmax).
  * attention in natural orientation: E tile is the STATIONARY operand
    ([128, 2, 128] fp8 DR slabs), moving [v | ones] fp8 pairs [128, 2, 65] ->
    psum [128 q-rows, 65] accumulated over all 32 key blocks. Output partition
    = row s of the chunk, so softmax divide + residual add is one fused
    scalar_tensor_tensor per 128x64 slab (denominator is per-partition).
  * LayerNorm on DVE (bn_stats + Newton rsqrt), gamma/beta ops elided when
    they are identity (they are for this model's inputs).
"""
import numpy as np
import ml_dtypes
from contextlib import ExitStack

_STATE = {}


def _imports():
    global bass, bacc, tile, mybir, bass_utils, F32, BF16, F8, U8, I32, ALU, ACTF, PM
    import concourse.bass as bass
    import concourse.bacc as bacc
    import concourse.tile as tile
    from concourse import mybir
    from concourse import bass_utils
    F32 = mybir.dt.float32
    BF16 = mybir.dt.bfloat16
    F8 = mybir.dt.float8e4
    U8 = mybir.dt.uint8
    I32 = mybir.dt.int32
    ALU = mybir.AluOpType
    ACTF = mybir.ActivationFunctionType
    PM = mybir.MatmulPerfMode


N_CORES = 8
CHUNKS_PER_CORE = 2
S = 512
D = 512
EPS = 1e-5
SHIFT = 20.0                       # exp((s-20)/8); cancels in softmax
A_SCH = 1.4426950408889634         # log2(e): u8 = round(A*s + C) -> e4m3 bits
C_SCH = 26.771099182220732         # 56 - 0.375 - A*20
N_DVE = 98                        # of 256 exp ops per core go to DVE
TOT_EXP = 256
DEBUG = False


def _emit(nc, tc, ctx, trivial_gb=True, zero_bv=True):
    x_d = nc.dram_tensor("xf", [CHUNKS_PER_CORE, S, D], F32, kind="ExternalInput").ap()
    xtb_d = nc.dram_tensor("xtb", [CHUNKS_PER_CORE, 128, 4 * D], BF16, kind="ExternalInput").ap()
    xt8_d = nc.dram_tensor("xt8", [CHUNKS_PER_CORE, 128, 4 * D], F8, kind="ExternalInput").ap()
    wq_d = nc.dram_tensor("wq", [128, 4 * D], BF16, kind="ExternalInput").ap()
    wk_d = nc.dram_tensor("wk", [128, 4 * D], BF16, kind="ExternalInput").ap()
    wv8_d = nc.dram_tensor("wv8", [128, 4 * D], F8, kind="ExternalInput").ap()
    bq_d = nc.dram_tensor("bq", [128, 4], F32, kind="ExternalInput").ap()
    bk_d = nc.dram_tensor("bk", [128, 4], F32, kind="ExternalInput").ap()
    y_d = nc.dram_tensor("y", [CHUNKS_PER_CORE, S, D], F32, kind="ExternalOutput").ap()
    if DEBUG:
        dbg_qT = nc.dram_tensor("dbg_qT", [128, 4 * D], BF16, kind="ExternalOutput").ap()
        dbg_qTs = nc.dram_tensor("dbg_qTs", [128, 4 * D], BF16, kind="ExternalOutput").ap()
        dbg_kT = nc.dram_tensor("dbg_kT", [128, 4 * D], BF16, kind="ExternalOutput").ap()
        dbg_vp = nc.dram_tensor("dbg_vp", [128, 2560], F8, kind="ExternalOutput").ap()
        dbg_e = nc.dram_tensor("dbg_e", [128, 1024], F8, kind="ExternalOutput").ap()
        dbg_pa = nc.dram_tensor("dbg_pa", [128, 320], F32, kind="ExternalOutput").ap()
        dbg_h = nc.dram_tensor("dbg_h", [128, 4 * D], F32, kind="ExternalOutput").ap()
    if not trivial_gb:
        gb_d = nc.dram_tensor("gb", [128, D], F32, kind="ExternalInput").ap()
        bb_d = nc.dram_tensor("bb", [128, D], F32, kind="ExternalInput").ap()
    if not zero_bv:
        ones8_d = nc.dram_tensor("ones8", [1, 128], F8, kind="ExternalInput").ap()
        bv8_d = nc.dram_tensor("bv8", [1, D], F8, kind="ExternalInput").ap()

    consts = ctx.enter_context(tc.tile_pool(name="consts", bufs=1))
    chunkp = ctx.enter_context(tc.tile_pool(name="chunk", bufs=2))
    epool = ctx.enter_context(tc.tile_pool(name="epool", bufs=24))
    small = ctx.enter_context(tc.tile_pool(name="small", bufs=4))
    ypool = ctx.enter_context(tc.tile_pool(name="ypool", bufs=2))
    ps_score = ctx.enter_context(tc.tile_pool(name="ps_score", bufs=3, space="PSUM"))
    ps_attn = ctx.enter_context(tc.tile_pool(name="ps_attn", bufs=1, space="PSUM"))
    ps_prep = ctx.enter_context(tc.tile_pool(name="ps_prep", bufs=1, space="PSUM"))

    wq = consts.tile([128, 4, D], BF16, tag="wq")
    wk = consts.tile([128, 4, D], BF16, tag="wk")
    wv8 = consts.tile([128, 4, D], F8, tag="wv8")
    bq = consts.tile([128, 4], F32, tag="bq")
    bk = consts.tile([128, 4], F32, tag="bk")
    if not trivial_gb:
        gb = consts.tile([128, D], F32, tag="gb")
        bb = consts.tile([128, D], F32, tag="bb")
    if not zero_bv:
        ones8 = consts.tile([1, 128], F8, tag="ones8")
        bv8 = consts.tile([1, D], F8, tag="bv8")

    ebias = consts.tile([128, 1], F32, tag="ebias")

    def consts_early():
        nc.vector.memset(ebias[:], -SHIFT / 8.0)
        wq4 = wq_d[:].rearrange("p (a b) -> p a b", a=4)
        nc.sync.dma_start(wq[:, :, 0:128], wq4[:, :, 0:128])

    def consts_mid():
        wk4 = wk_d[:].rearrange("p (a b) -> p a b", a=4)
        nc.sync.dma_start(wk[:, :, 0:128], wk4[:, :, 0:128])
        nc.sync.dma_start(bq[:], bq_d[:])
        nc.sync.dma_start(bk[:], bk_d[:])
        wq4 = wq_d[:].rearrange("p (a b) -> p a b", a=4)
        nc.sync.dma_start(wq[:, :, 128:512], wq4[:, :, 128:512])
        nc.sync.dma_start(wk[:, :, 128:512], wk4[:, :, 128:512])

    def consts_late():
        # xt8(0) deferred here: only the v-projection needs it, and it must
        # not delay wk-dt0 on the startup-critical DMA queue.
        nc.sync.dma_start(st[0]["xt8"][:], xt8_d[0].rearrange("p (a b) -> p a b", a=4))
        nc.sync.dma_start(wv8[:], wv8_d[:].rearrange("p (a b) -> p a b", a=4))
        if not trivial_gb:
            nc.sync.dma_start(gb[:], gb_d[:])
            nc.sync.dma_start(bb[:], bb_d[:])
        if not zero_bv:
            nc.sync.dma_start(ones8[:], ones8_d[:])
            nc.sync.dma_start(bv8[:], bv8_d[:])

    st = [{} for _ in range(CHUNKS_PER_CORE)]

    def prep_load(c, xf_too=True):
        s_ = st[c]
        s_["xtb"] = chunkp.tile([128, 4, D], BF16, tag="xtb", name=f"xtb{c}")
        s_["xt8"] = chunkp.tile([128, 4, D], F8, tag="xt8", name=f"xt8{c}")
        s_["xf"] = chunkp.tile([128, 4, D], F32, tag="xf", name=f"xf{c}")
        xtb4 = xtb_d[c].rearrange("p (a b) -> p a b", a=4)
        nc.sync.dma_start(s_["xtb"][:, 0:2, :], xtb4[:, 0:2, :])
        nc.sync.dma_start(s_["xtb"][:, 2:4, :], xtb4[:, 2:4, :])
        if c != 0:
            nc.sync.dma_start(s_["xt8"][:], xt8_d[c].rearrange("p (a b) -> p a b", a=4))
        s_["qT"] = chunkp.tile([128, 4, D], BF16, tag="qT", name=f"qT{c}")
        s_["qTs"] = chunkp.tile([128, 4, D], BF16, tag="qTs", name=f"qTs{c}")
        s_["kT"] = chunkp.tile([128, 4, D], BF16, tag="kT", name=f"kT{c}")
        s_["vp8"] = chunkp.tile([128, 4, 4, 2, 80], F8, tag="vp8", name=f"vp8{c}")
        s_["h"] = chunkp.tile([128, 4, D], F32, tag="h", name=f"h{c}")
        s_["st6"] = chunkp.tile([128, 4, 8, 6], F32, tag="st6", name=f"st6{c}")
        if xf_too:
            prep_load_xf(c)

    def prep_load_xf(c):
        s_ = st[c]
        for t in range(4):
            nc.sync.dma_start(s_["xf"][:, t, :], x_d[c, 128 * t:128 * (t + 1), :])

    def prep_qk_mm(c, dt, which):
        """Matmul half of a q^T/k^T projection tile; returns the psum AP."""
        xtb = st[c]["xtb"]
        w = wq if which == "q" else wk
        pp = ps_prep.tile([128, D], F32, tag="prep",
                          name=f"pp{c}_{which}{dt}")
        for mt in range(4):
            nc.tensor.matmul(pp[:], w[:, mt, 128 * dt:128 * (dt + 1)],
                             xtb[:, mt, :], start=(mt == 0), stop=(mt == 3))
        return pp

    def prep_qk_copy(c, dt, which, pp, swap_eng=None):
        """Copy half: psum -> bf16 (+bias), and the qTs quadrant swap.
        The swap DMA goes on the (otherwise idle) Pool queue so it cannot
        head-of-line block the SP bulk-load queue; the startup-critical one
        rides the DVE queue right behind its producer copy."""
        s_ = st[c]
        bias, dst = ((bq, s_["qT"]) if which == "q" else (bk, s_["kT"]))
        nc.vector.tensor_scalar(dst[:, dt, :], pp[:], bias[:, dt:dt + 1], None,
                                op0=ALU.add)
        if which == "q":
            qTs = s_["qTs"]
            eng = swap_eng or nc.sync
            eng.dma_start(qTs[64:128, dt, :], dst[0:64, dt, :])
            eng.dma_start(qTs[0:64, dt, :], dst[64:128, dt, :])

    def prep_v_mm(c, stt_):
        """Matmul half of a v s-tile (fp8 DoubleRow); returns the psum AP."""
        xt4 = st[c]["xt8"][:].rearrange("p (u j) s -> p u j s", u=2)
        wv4 = wv8[:].rearrange("p (u j) s -> p u j s", u=2)
        pv = ps_prep.tile([128, D], F32, tag="prep",
                          name=f"pv{c}_{stt_}")
        for u in range(2):
            nc.tensor.matmul(pv[:], xt4[:, u, :, 128 * stt_:128 * (stt_ + 1)],
                             wv4[:, u, :, :],
                             start=(u == 0), stop=(u == 1) and zero_bv,
                             perf_mode=PM.DoubleRow)
        if not zero_bv:
            nc.tensor.matmul(pv[:], ones8[:], bv8[:], start=False, stop=True)
        return pv

    def prep_v_copy(c, stt_, pv):
        """Copy half: [v | 1] pairs into vp8 (fp8)."""
        vp = st[c]["vp8"]
        nc.vector.tensor_copy(
            vp[:, stt_, :, :, 0:64],
            pv[:].rearrange("p (a b c) -> p a b c", a=4, b=2))
        nc.vector.memset(vp[:, stt_, :, :, 64], 1.0)

    def qrhs(c, jq, par):
        src = st[c]["qT"] if (jq % 2) == par else st[c]["qTs"]
        return src[64 * par:64 * par + 64, jq // 2, :]

    def strip_scores(c, jq, r, jku, gctr, use_dve):
        """Scores pair + exp for one (jq, r, jku); returns the e tile."""
        s_ = st[c]
        kT = s_["kT"]
        pspair = ps_score.tile([128, 2, D], F32, tag="sps",
                               name=f"sp{c}_{jq}_{r}_{jku}")
        nc.tensor.matmul(pspair[:, 0, :], kT[0:64, jku, 128 * r:128 * r + 128],
                         qrhs(c, jq, 0), start=True, stop=True,
                         tile_position=(0, 0))
        nc.tensor.matmul(pspair[:, 1, :], kT[64:128, jku, 128 * r:128 * r + 128],
                         qrhs(c, jq, 1), start=True, stop=True,
                         tile_position=(64, 0))
        e = epool.tile([128, 2, D], F8, tag="e", name=f"e{c}_{jq}_{r}_{jku}")
        if use_dve:
            nc.vector.tensor_scalar(e[:].bitcast(U8), pspair[:], A_SCH, C_SCH,
                                    op0=ALU.mult, op1=ALU.add)
        else:
            nc.scalar.activation(e[:], pspair[:], ACTF.Exp,
                                 bias=ebias[:], scale=0.125)
        return e

    def attn_burst(c, jq, b, pa, e_list):
        """One contiguous DR accumulation burst: pa slab b over all 16
        (jku, r) key blocks of this jq. Groups must NOT be interleaved:
        interleaved fp8-DoubleRow accumulation groups corrupt on TRN2."""
        vp = st[c]["vp8"]
        for t in range(16):
            jku, r = t // 4, t % 4
            nc.tensor.matmul(pa[:, b, 0:65],
                             e_list[t][:, :, 128 * b:128 * (b + 1)],
                             vp[:, r, jku, :, 0:65],
                             start=(t == 0), stop=(t == 15),
                             perf_mode=PM.DoubleRow, skip_group_check=True)

    def fin_pieces(c, jq, pa):
        """Finalize micro-pieces (one per strip slot): softmax divide +
        residual add + incremental LN stats, per 128x64 slab."""
        s_ = st[c]
        h, xf, st6 = s_["h"], s_["xf"], s_["st6"]
        rcp = small.tile([128, 4], F32, tag="rcp", name=f"rcp{c}_{jq}")

        def p_rcp():
            nc.vector.reciprocal(rcp[:], pa[:, :, 64])

        def p_slab(b):
            nc.vector.scalar_tensor_tensor(
                h[:, b, 64 * jq:64 * jq + 64],
                pa[:, b, 0:64], rcp[:, b:b + 1],
                xf[:, b, 64 * jq:64 * jq + 64],
                op0=ALU.mult, op1=ALU.add)
            nc.vector.bn_stats(st6[:, b, jq, :], h[:, b, 64 * jq:64 * jq + 64])

        return [p_rcp] + [lambda b=b: p_slab(b) for b in range(4)]

    def ln_pieces(c, tail=False):
        """LayerNorm micro-pieces; Newton rsqrt batched across the 4 s-tiles."""
        s_ = st[c]
        h = s_["h"]
        mvall = small.tile([128, 8], F32, tag="mvall", name=f"mv{c}")
        st6 = s_["st6"]
        t4 = small.tile([128, 4], F32, tag="t4", name=f"t4_{c}")
        yi = small.tile([128, 4], I32, tag="yi", name=f"yi{c}")
        rstd = small.tile([128, 4], F32, tag="rstd", name=f"rstd{c}")
        y2 = small.tile([128, 4], F32, tag="y2", name=f"y2_{c}")
        dd = small.tile([128, 4], F32, tag="dd", name=f"dd{c}")
        bco = small.tile([128, 4], F32, tag="bco", name=f"bco{c}")
        yt = ypool.tile([128, 4, D], F32, tag="yt", name=f"yt{c}")
        mean4 = mvall[:].rearrange("p (b two) -> p b two", two=2)[:, :, 0]
        var4 = mvall[:].rearrange("p (b two) -> p b two", two=2)[:, :, 1]
        ps = []

        def p_aggr(b):
            nc.vector.bn_aggr(mvall[:, 2 * b:2 * b + 2],
                              st6[:, b, :, :].rearrange("p a b -> p (a b)"))
        ps += [lambda b=b: p_aggr(b) for b in range(4)]

        def p_seed():
            nc.vector.tensor_scalar_add(t4[:], var4, EPS)
            nc.vector.tensor_scalar(yi[:], t4[:].bitcast(I32), 1, None,
                                    op0=ALU.arith_shift_right)
            nc.vector.tensor_scalar(yi[:], yi[:], 0x5F3759DF, -1,
                                    op0=ALU.subtract, op1=ALU.mult)
            nc.vector.tensor_copy(rstd[:], yi[:].bitcast(F32))
        ps.append(p_seed)

        def p_newton():
            nc.vector.tensor_tensor(y2[:], rstd[:], rstd[:], op=ALU.mult)
            nc.vector.tensor_tensor(y2[:], y2[:], t4[:], op=ALU.mult)
            nc.vector.tensor_scalar(dd[:], y2[:], -0.5, 1.5,
                                    op0=ALU.mult, op1=ALU.add)
            nc.vector.tensor_tensor(rstd[:], rstd[:], dd[:], op=ALU.mult)
        ps += [p_newton, p_newton]

        def p_bco():
            nc.vector.tensor_tensor(bco[:], mean4, rstd[:], op=ALU.mult)
            nc.vector.tensor_scalar_mul(bco[:], bco[:], -1.0)
        ps.append(p_bco)

        def p_yt(b):
            nc.vector.tensor_scalar(yt[:, b, :], h[:, b, :],
                                    rstd[:, b:b + 1], bco[:, b:b + 1],
                                    op0=ALU.mult, op1=ALU.add)
            if not trivial_gb:
                nc.vector.tensor_tensor(yt[:, b, :], yt[:, b, :], gb[:], op=ALU.mult)
                nc.vector.tensor_tensor(yt[:, b, :], yt[:, b, :], bb[:], op=ALU.add)
        ps += [lambda b=b: p_yt(b) for b in range(4)]

        def p_dma():
            nc.sync.dma_start(y_d[c].rearrange("(t p) d -> p t d", t=4), yt[:])
        ps.append(p_dma)
        return ps

    # ---- emission schedule -------------------------------------------------
    # Minimal prep before strips start (weights+x DMAs, q0/k0/v0); everything
    # else is fed into strip slots as split (matmul, copy) pieces so a stolen
    # score-psum buffer is returned quickly. Strips iterate jku-outer so the
    # first slots only need kT tile 0. attn is emitted ATTN_DELAY slots late
    # (in-order PE queue must not park on exp); finalize 2 more slots late.
    consts_early()
    prep_load(0, xf_too=False)
    consts_mid()
    pp = prep_qk_mm(0, 0, "q"); prep_qk_copy(0, 0, "q", pp)
    pp = prep_qk_mm(0, 0, "k"); prep_qk_copy(0, 0, "k", pp)
    consts_late()
    for dt in (1, 2, 3):
        pp = prep_qk_mm(0, dt, "k"); prep_qk_copy(0, dt, "k", pp)
    pv = prep_v_mm(0, 0); prep_v_copy(0, 0, pv)
    prep_load_xf(0)

    iters = []
    for c in range(CHUNKS_PER_CORE):
        for jq in range(8):
            for jku in range(4):
                for r in range(4):
                    iters.append((c, jq, r, jku))
    n_slots = len(iters)

    use_dve_at = [(((g + 1) * N_DVE) // TOT_EXP != (g * N_DVE) // TOT_EXP)
                  for g in range(TOT_EXP)]

    # In-stream prep pieces. EMISSION ORDER IS THE DEPENDENCY ORDER: a piece's
    # copy pops at the START of the following slot, so it must precede the
    # first strip instruction that reads its output. v(0,st) is read by the
    # attn of slot (st) which is emitted at slot st+2 -> mm at slot st keeps
    # the copy one slot ahead. Everything else has slack; those steal slots
    # are nudged so the NEXT slot's exp runs on ACT (the prep copy on DVE then
    # overlaps it instead of delaying a DVE exp).
    prep_at = {1: ("v", 0, 1), 2: ("v", 0, 2), 3: ("v", 0, 3)}
    pieces = [("q", 0, 1), ("q", 0, 2), ("q", 0, 3), ("load", 1, 0)]
    for dt in range(4):
        pieces.append(("q", 1, dt))
        pieces.append(("k", 1, dt))
    for stt_ in range(4):
        pieces.append(("v", 1, stt_))
    slot = 6
    for p in pieces:
        while slot + 1 < TOT_EXP and (use_dve_at[slot + 1] or slot in prep_at):
            slot += 1
        prep_at[slot] = p
        slot += 6
    assert slot < 128

    pa_tiles = {}
    pending_attn = []
    pending_fin = []
    pending_copy = []
    dve_q = []
    ATTN_DELAY = 2
    gctr = 0
    for si, (c, jq, r, jku) in enumerate(iters):
        if (c, jq) not in pa_tiles:
            pa_tiles[(c, jq)] = ps_attn.tile([128, 4, 80], F32, tag="pa",
                                             name=f"pa{c}_{jq}")
        if DEBUG and si == 0:
            s0 = st[0]
            nc.sync.dma_start(dbg_qT[:].rearrange("p (a b) -> p a b", a=4), s0["qT"][:])
            nc.sync.dma_start(dbg_qTs[:].rearrange("p (a b) -> p a b", a=4), s0["qTs"][:])
            nc.sync.dma_start(dbg_kT[:].rearrange("p (a b) -> p a b", a=4), s0["kT"][:])
        if DEBUG and si == 20:
            s0 = st[0]
            nc.sync.dma_start(dbg_vp[:].rearrange("p (a b c d) -> p a b c d", a=4, b=4, c=2), s0["vp8"][:])
        if pending_copy:
            kind, pc2, idx, ap = pending_copy.pop(0)
            if kind == "v":
                prep_v_copy(pc2, idx, ap)
            else:
                prep_qk_copy(pc2, idx, kind, ap)
        e = strip_scores(c, jq, r, jku, gctr, use_dve_at[gctr])
        if DEBUG and si == 0:
            nc.sync.dma_start(dbg_e[:].rearrange("p (a b) -> p a b", a=2), e[:])
        if len(pending_attn) >= ATTN_DELAY:
            pc, pj, pr, pk, pe = pending_attn.pop(0)
            strip_attn(pc, pj, pr, pk, pa_tiles[(pc, pj)], pe)
            if pr == 3 and pk == 3:
                pending_fin.append([pc, pj, 2])
        pending_attn.append((c, jq, r, jku, e))
        gctr += 1
        for f in pending_fin:
            f[2] -= 1
        while pending_fin and pending_fin[0][2] <= 0:
            fc, fj, _ = pending_fin.pop(0)
            if DEBUG and fc == 0 and fj == 0:
                stg = small.tile([128, 4, 80], F32, tag="dbgstg")
                nc.vector.tensor_copy(stg[:], pa_tiles[(fc, fj)][:])
                nc.sync.dma_start(dbg_pa[:].rearrange("p (a b) -> p a b", a=4),
                                  stg[:])
            dve_q.extend(fin_pieces(fc, fj, pa_tiles.pop((fc, fj))))
            if fj == 7:
                if DEBUG and fc == 0:
                    dve_q.append(lambda: nc.sync.dma_start(
                        dbg_h[:].rearrange("p (a b) -> p a b", a=4), st[0]["h"][:]))
                dve_q.extend(ln_pieces(fc))
        if dve_q:
            dve_q.pop(0)()
        piece = prep_at.get(si)
        if piece is not None:
            kind, pc2, idx = piece
            if kind == "load":
                prep_load(1)
            elif kind == "v":
                pending_copy.append(("v", pc2, idx, prep_v_mm(pc2, idx)))
            else:
                pending_copy.append((kind, pc2, idx, prep_qk_mm(pc2, idx, kind)))
    for pc, pj, pr, pk, pe in pending_attn:
        strip_attn(pc, pj, pr, pk, pa_tiles[(pc, pj)], pe)
        if pr == 3 and pk == 3:
            pending_fin.append([pc, pj, 0])
    for fc, fj, _ in pending_fin:
        dve_q.extend(fin_pieces(fc, fj, pa_tiles.pop((fc, fj))))
        if fj == 7:
            dve_q.extend(ln_pieces(fc))
    while dve_q:
        dve_q.pop(0)()


def build(trivial_gb=True, zero_bv=True):
    key = ("nc", trivial_gb, zero_bv)
    if key in _STATE:
        return _STATE[key]
    _imports()
    nc = bacc.Bacc("TRN2", target_bir_lowering=False, debug=False,
                   num_devices=N_CORES)
    with tile.TileContext(nc) as tc:
        with ExitStack() as ctx:
            _emit(nc, tc, ctx, trivial_gb=trivial_gb, zero_bv=zero_bv)
    nc.compile()
    _STATE[key] = nc
    return nc


def kernel(x, Wq, bq, Wk, bk, Wv, bv, gamma, beta):
    _imports()
    bf = ml_dtypes.bfloat16
    f8 = ml_dtypes.float8_e4m3fn
    x = np.asarray(x, np.float32)
    Wq = np.asarray(Wq, np.float32)
    Wk = np.asarray(Wk, np.float32)
    Wv = np.asarray(Wv, np.float32)
    bq = np.asarray(bq, np.float32)
    bk = np.asarray(bk, np.float32)
    bv = np.asarray(bv, np.float32)
    gamma = np.asarray(gamma, np.float32)
    beta = np.asarray(beta, np.float32)
    trivial_gb = bool(np.all(gamma == 1.0) and np.all(beta == 0.0))
    zero_bv = bool(np.all(bv == 0.0))
    nc = build(trivial_gb=trivial_gb, zero_bv=zero_bv)

    B, Sfull, Dm = x.shape
    chunks = x.reshape(B * 8, S, D)
    chT = np.ascontiguousarray(chunks.transpose(0, 2, 1))          # [16, 512m, 512s]
    xtb = np.ascontiguousarray(
        chT.reshape(B * 8, 4, 128, D).transpose(0, 2, 1, 3)
    ).reshape(B * 8, 128, 4 * D).astype(bf)
    xt8 = np.ascontiguousarray(
        chT.reshape(B * 8, 2, 2, 128, D).transpose(0, 3, 1, 2, 4)
    ).reshape(B * 8, 128, 4 * D).astype(f8)

    base = {
        "wq": np.ascontiguousarray(
            Wq.reshape(4, 128, D).transpose(1, 0, 2)).reshape(128, 4 * D).astype(bf),
        "wk": np.ascontiguousarray(
            Wk.reshape(4, 128, D).transpose(1, 0, 2)).reshape(128, 4 * D).astype(bf),
        "wv8": np.ascontiguousarray(
            Wv.reshape(2, 2, 128, D).transpose(2, 0, 1, 3)).reshape(128, 4 * D).astype(f8),
        "bq": np.ascontiguousarray(bq.reshape(4, 128).T),
        "bk": np.ascontiguousarray(bk.reshape(4, 128).T),
    }
    if not trivial_gb:
        base["gb"] = np.broadcast_to(gamma, (128, D)).copy()
        base["bb"] = np.broadcast_to(beta, (128, D)).copy()
    if not zero_bv:
        base["ones8"] = np.ones((1, 128), f8)
        base["bv8"] = bv.reshape(1, D).astype(f8)

    in_maps = []
    for i in range(N_CORES):
        m = dict(base)
        m["xf"] = np.ascontiguousarray(chunks[2 * i:2 * i + 2])
        m["xtb"] = np.ascontiguousarray(xtb[2 * i:2 * i + 2])
        m["xt8"] = np.ascontiguousarray(xt8[2 * i:2 * i + 2])
        in_maps.append(m)
    res = bass_utils.run_bass_kernel_spmd(nc, in_maps, core_ids=list(range(N_CORES)))
    out = np.empty((B * 8, S, D), np.float32)
    for i in range(N_CORES):
        out[2 * i:2 * i + 2] = res.results[i]["y"]
    return out.reshape(B, Sfull, Dm)
